# revision 2
# baseline (speedup 1.0000x reference)
"""Paged-KV varlen causal GQA attention for Trainium2, sharded over 8 NeuronCores.

Problem (hardcoded from spec): T=4096 tokens, 16 q heads / 8 kv heads, head_dim=64,
scale=0.125. k/v are scattered into paged caches via slot_mapping, read back, and
causal varlen attention (segments from cu_seqlens) is computed.

Sharding: tensor-parallel over kv heads -- core h gets kv head h and its 2 GQA
query heads. slot_mapping / cu_seqlens handled on host (index math only).

Device kernel (per core), per (segment, q-head), score layout sT[keys, queries]:
  scores are packed tightly into wide PSUM tiles (up to 1536 f32 cols = 3 banks)
  so the ScalarE exp -- the serial bottleneck at 1 col/cycle -- runs in as few,
  as wide instructions as possible.
    sT[k, q] = kT.T @ qT                (PE, per 128-key tile, causal spans only)
    sT[diag] += Mtri.T @ I              (PE; adds -240 above the diagonal so the
                                         exp kills masked entries -- no VectorE
                                         mask multiply needed)
    w = exp(0.125 * sT)                 (ScalarE, PSUM->SBUF bf16, one exp/tile)
    o_acc[q, 0:65] += w.T @ [v | 1]     (PE: w slices are the stationary weights,
                                         65-wide moving v => 65 cycles per
                                         (key-tile, 128-query block); col 64 is
                                         the softmax denominator)
    o[q, :] = o_acc[q, 0:64] * rcp(den) (VectorE reciprocal + broadcast mul)
"""

import os
from math import ceil

import numpy as np
import ml_dtypes

import concourse.bass as bass
import concourse.mybir as mybir
import concourse.tile as tile
from concourse import bacc
from concourse.bass_utils import run_bass_kernel_spmd
from concourse.masks import make_identity

NKV = 8
G = 2
D = 64
SCALE = 0.125
NEG = -240.0  # NEG*SCALE = -30; exp(-30) ~ 1e-13

TRACE = bool(int(os.environ.get("KERNEL_TRACE", "0")))
LAST_RESULT = None

_PROGRAM_CACHE = {}


def _pack_tiles(L, widths):
    """Pack the causal spans of all 128-key tiles of one (segment, head) into
    score tiles of the given widths. Returns [(width, [(kt, qlo, qhi, off)])].
    All quantities are multiples of 128."""
    total = sum(L - 128 * kt for kt in range(L // 128))
    assert sum(widths) == total, (sum(widths), total)
    tiles = []
    cur = []
    used = 0
    wi = 0
    for kt in range(L // 128):
        pos = 128 * kt
        while pos < L:
            take = min(L - pos, widths[wi] - used)
            cur.append((kt, pos, pos + take, used))
            used += take
            pos += take
            if used == widths[wi]:
                tiles.append((widths[wi], cur))
                cur = []
                used = 0
                wi += 1
    assert not cur
    return tiles


def _build_program(T, segments):
    f32 = mybir.dt.float32
    bf16 = mybir.dt.bfloat16
    Exp = mybir.ActivationFunctionType.Exp

    nc = bacc.Bacc(
        "TRN2",
        target_bir_lowering=False,
        debug=False,
        enable_asserts=False,
        num_devices=8,
    )
    # qk_d[:, 0:T] = qT (two heads stacked on partitions); qk_d[:, T:2T] = kT
    # duplicated on both partition halves -- one DMA covers a head's q AND k
    qk_d = nc.dram_tensor("qk", [128, 2 * T], bf16, kind="ExternalInput").ap()
    v_d = nc.dram_tensor("v", [T, D], bf16, kind="ExternalInput").ap()
    o_d = nc.dram_tensor("o", [T, 2 * D], f32, kind="ExternalOutput").ap()

    NKT = T // 128  # key tiles over all segments

    with tile.TileContext(nc) as tc:
        import contextlib

        ctx = contextlib.ExitStack()
        const = ctx.enter_context(tc.tile_pool(name="const", bufs=1))
        qkpool = ctx.enter_context(tc.tile_pool(name="qk", bufs=1))
        vpool = ctx.enter_context(tc.tile_pool(name="vt", bufs=1))
        spool = ctx.enter_context(tc.tile_pool(name="se", bufs=7))
        opool = ctx.enter_context(tc.tile_pool(name="ob", bufs=2))
        ps_sc = ctx.enter_context(tc.tile_pool(name="ps_sc", bufs=2, space="PSUM"))
        ps_o = ctx.enter_context(tc.tile_pool(name="ps_o", bufs=1, space="PSUM"))

        # warmup activation: forces the Exp table load at t~0, overlapping the
        # input DMAs instead of delaying the first real exp
        wsrc = const.tile([128, 8], bf16)
        nc.gpsimd.memset(wsrc, 0.0)
        nc.scalar.activation(wsrc, wsrc, Exp, scale=SCALE)

        # Mtri[r, c] = NEG if c > r else 0; accumulated into the diagonal
        # 128-block of each score tile via I128 it adds NEG above the diagonal
        Mtri = const.tile([128, 128], bf16)
        nc.gpsimd.memset(Mtri, 0.0)
        nc.gpsimd.affine_select(
            out=Mtri,
            in_=Mtri,
            compare_op=mybir.AluOpType.is_gt,
            fill=NEG,
            base=1,
            pattern=[[-1, 128]],
            channel_multiplier=1,
        )
        I128 = const.tile([128, 128], bf16)
        make_identity(nc, I128)

        # qkt[:, 0:T] = qT; qkt[:, T:2T] = kT duplicated on both halves so
        # head h's QK contracts on PE rows [64h, 64h+64) with matching
        # lhsT/rhs base partitions
        qkt = qkpool.tile([128, 2 * T], bf16)
        qT = qkt[:, 0:T]
        kT = qkt[:, T : 2 * T]
        vst = vpool.tile([128, NKT, D + 1], bf16)

        def dma_qk(rows, c0, c1, eng=None):
            # one DMA for q cols [c0,c1) AND k cols [c0,c1) of a head half
            # (regular 3-level AP: two chunks at stride T)
            r0, r1 = rows
            (eng or nc.sync).dma_start(
                qkt[r0:r1].rearrange("p (c n) -> p c n", c=2)[:, :, c0:c1],
                qk_d[r0:r1].rearrange("p (c n) -> p c n", c=2)[:, :, c0:c1],
            )

        # --- input DMAs, all upfront (dep tracking is byte-range based) ---
        for si, (s0, s1) in enumerate(segments):
            if si == 0 and s1 - s0 >= 1024:
                # ladder: tiny k then q chunks so the first QK unblocks ~2.9us
                # in instead of waiting for full-segment transfers
                # laddered q+k chunks: the first exp needs only chunk 1
                dma_qk((0, 64), s0, s0 + 256)
                dma_qk((0, 64), s0 + 256, s0 + 768)
                dma_qk((0, 64), s0 + 768, s1)
                g0 = s0 // 128
                ng = (s1 - s0) // 128
                # v and head 1 ride the Pool SWDGE queue: its generation runs
                # on the idle Pool engine, landing them ~2us earlier than
                # queueing behind head 0's transfers on SP
                nc.gpsimd.dma_start(
                    vst[:, g0 : g0 + ng, 0:D],
                    v_d[s0:s1, :].rearrange("(n p) d -> p n d", p=128),
                )
                dma_qk((64, 128), s0, s1, eng=nc.gpsimd)
            else:
                g0 = s0 // 128
                ng = (s1 - s0) // 128
                dma_qk((0, 64), s0, s1)
                # non-critical later transfers ride the idle Pool SWDGE queue
                # (input DMAs have no waits, so its sequencer holds are short)
                dma_qk((64, 128), s0, s1, eng=nc.gpsimd)
                nc.gpsimd.dma_start(
                    vst[:, g0 : g0 + ng, 0:D],
                    v_d[s0:s1, :].rearrange("(n p) d -> p n d", p=128),
                )
            nc.gpsimd.memset(vst[:, g0 : g0 + ng, D : D + 1], 1.0)

        # --- work list: one entry per score tile ---
        work = []
        for si, (s0, s1) in enumerate(segments):
            L = s1 - s0
            assert L % 128 == 0
            total = sum(L - 128 * kt for kt in range(L // 128))
            for h in range(G):
                first_it = si == 0 and h == 0
                last_it = si == len(segments) - 1 and h == G - 1
                if first_it and L == 1024:
                    # ramp: tile 0 (all of kt0) gets four 256-wide sub-exps --
                    # the first one only needs the first small q/k DMA, so the
                    # ScalarE stream starts ~1us earlier; sub-exps share one
                    # PSUM buffer (disjoint column ranges, no rotation WAR)
                    widths = [1024, 640, 1408, 1536]
                    assert sum(widths) == total
                elif last_it and L == 1024:
                    # ramp down: kt6/kt7 diagonals land in narrow final tiles
                    # so the closing exp->PV->normalize->DMA chain is short
                    widths = [1536, 1536, 1024, 384, 128]
                else:
                    widths = []
                rem = total - sum(widths)
                while rem > 0:
                    w = min(1536, rem)
                    widths.append(w)
                    rem -= w
                work.append((si, s0, L, h, _pack_tiles(L, widths)))

        state = {}  # (si, h) -> dict(oacc=, rcp=)
        osb_by_seg = {}

        def emit_qk(si, s0, h, ti, width, pieces, splits=None):
            sc = ps_sc.tile([128, 1536], f32, tag="sc", name=f"sc_{si}_{h}_{ti}")
            se = spool.tile([128, 1536], bf16, tag="se", name=f"se_{si}_{h}_{ti}")
            sa = 0
            for sw in splits or [width]:
                sb = sa + sw
                for kt, qlo, qhi, off in pieces:
                    # clip the piece to this sub-exp's column range [sa, sb)
                    a = max(off, sa)
                    b = min(off + (qhi - qlo), sb)
                    if a >= b:
                        continue
                    klo = 128 * kt
                    diag = qlo == klo and a == off
                    c = a
                    while c < b:
                        # chunks split at PSUM bank boundaries AND at the end
                        # of the diagonal block, so each accumulation group's
                        # region is closed exactly by its last matmul
                        e = min(b, (c // 512 + 1) * 512)
                        is_diag = diag and c == a
                        if is_diag:
                            e = min(e, off + 128)
                        qa = s0 + qlo + (c - off)
                        nc.tensor.matmul(
                            sc[:, c:e],
                            kT[64 * h : 64 * h + 64, s0 + klo : s0 + klo + 128],
                            qT[64 * h : 64 * h + 64, qa : qa + (e - c)],
                            start=True,
                            stop=not is_diag,
                        )
                        if is_diag:
                            nc.tensor.matmul(
                                sc[:, off : off + 128],
                                Mtri[:, 0:128],
                                I128[:, 0:128],
                                start=False,
                                stop=True,
                            )
                        c = e
                nc.scalar.activation(se[:, sa:sb], sc[:, sa:sb], Exp, scale=SCALE)
                sa = sb
            return se

        def fin(si, s0, L, h, q0, nq):
            """Normalize query blocks [q0, q0+nq) and, on the last head, DMA
            those 128*nq output rows (both heads' columns) to HBM."""
            st = state[(si, h)]
            oacc = st["oacc"]
            if st["rcp"] is None:
                st["rcp"] = opool.tile([128, 8], f32, tag="rcp", name=f"rcp_{si}_{h}")
            rcp = st["rcp"]
            if si not in osb_by_seg:
                osb_by_seg[si] = opool.tile(
                    [128, 8, 2 * D], f32, tag="osb", name=f"osb_{si}"
                )
            osb = osb_by_seg[si]
            ot = oacc[q0 // 4]
            lo = 128 * (q0 % 4)
            nc.vector.reciprocal(
                rcp[:, q0 : q0 + nq],
                ot[:, lo + D : lo + D + 128 * (nq - 1) + 1 : 128],
            )
            ov = ot[:, lo : lo + 128 * nq].rearrange("p (c d) -> p c d", d=128)[
                :, :, 0:D
            ]
            rv = rcp[:, q0 : q0 + nq].rearrange("p (c d) -> p c d", d=1)
            rv2, ov2 = bass.broadcast_tensor_aps(rv, ov)
            nc.vector.tensor_mul(osb[:, q0 : q0 + nq, D * h : D * h + D], ov2, rv2)
            if h == G - 1:
                nc.sync.dma_start(
                    o_d[s0 + 128 * q0 : s0 + 128 * (q0 + nq), :].rearrange(
                        "(c p) d -> p c d", p=128
                    ),
                    osb[:, q0 : q0 + nq, :],
                )

        def emit_pv_block(it_key, qt):
            """All PV matmuls for query block qt of iteration it_key, in kt
            order -- each PSUM bank sees strictly sequential accumulation
            groups (one open group per bank, a hardware/CoreSim constraint)."""
            si, s0, L, h, pieces_of, se_of = it_key
            key = (si, h)
            if key not in state:
                state[key] = {
                    "oacc": [
                        ps_o.tile(
                            [128, 512], f32, tag=f"oacc{j}", name=f"oacc_{si}_{h}_{j}"
                        )
                        for j in range((L // 128 + 3) // 4)
                    ],
                    "rcp": None,
                }
            oacc = state[key]["oacc"]
            g0 = s0 // 128
            nq_total = L // 128
            out = oacc[qt // 4][:, 128 * (qt % 4) : 128 * (qt % 4) + D + 1]
            for kt in range(qt + 1):
                gti, qlo, off = pieces_of[(kt, qt)]
                rel = off + (128 * qt - qlo)
                nc.tensor.matmul(
                    out,
                    se_of[gti][:, rel : rel + 128],
                    vst[:, g0 + kt, :],
                    start=(kt == 0),
                    stop=(kt == qt),
                )
            if qt % 4 == 3 or qt == nq_total - 1:
                q0 = (qt // 4) * 4
                fin(si, s0, L, h, q0, min(4, nq_total - q0))

        # Emission: QK+exp stream per score tile; each query block's PV burst
        # is emitted LAG tiles after the tile holding its diagonal, so by the
        # time PV instructions reach the PE sequencer their exp has long
        # completed -- they never camp in the 4-deep PE wait queue blocking
        # the next QK's dispatch, and the ScalarE exp stream stays gapless.
        LAG = 3
        pending = []  # (global_tile_idx_of_diag, it_key, qt)
        gidx = 0
        for si, s0, L, h, tiles in work:
            pieces_of = {}
            diag_tile = {}
            for ti, (width, pieces) in enumerate(tiles):
                for kt, qlo, qhi, off in pieces:
                    for qt in range(qlo // 128, qhi // 128):
                        pieces_of[(kt, qt)] = (gidx + ti, qlo, off)
                    if qlo == 128 * kt:
                        diag_tile[kt] = gidx + ti
            se_of = {}
            it_key = (si, s0, L, h, pieces_of, se_of)
            for ti, (width, pieces) in enumerate(tiles):
                splits = (
                    [256, 256, 256, 256]
                    if (si == 0 and h == 0 and ti == 0 and width == 1024)
                    else None
                )
                se_of[gidx + ti] = emit_qk(si, s0, h, ti, width, pieces, splits)
                for qt in sorted(diag_tile):
                    if diag_tile[qt] == gidx + ti:
                        pending.append((gidx + ti, it_key, qt))
                while pending and pending[0][0] <= gidx + ti - LAG:
                    _, ik, qt = pending.pop(0)
                    emit_pv_block(ik, qt)
            gidx += len(tiles)
        for _, ik, qt in pending:
            emit_pv_block(ik, qt)

        ctx.close()

    nc.compile()
    return nc


def _segments_from_cu(cu_seqlens, T):
    edges = sorted(set([0, T] + [int(c) for c in cu_seqlens if 0 < int(c) < T]))
    return [(edges[i], edges[i + 1]) for i in range(len(edges) - 1)]


def kernel(q, k, v, k_cache, v_cache, slot_mapping, cu_seqlens):
    global LAST_RESULT
    T = q.shape[0]
    nslots = k_cache.shape[0]

    # Emulate scatter-then-gather through the paged cache: for duplicate slots
    # the last writer wins, so token i reads back k[lastw[slot[i]]].
    slot = np.asarray(slot_mapping, dtype=np.int64)
    lastw = np.zeros(nslots, dtype=np.int64)
    lastw[slot] = np.arange(T)
    lw = lastw[slot]
    k_eff = np.asarray(k)[lw]
    v_eff = np.asarray(v)[lw]

    segments = _segments_from_cu(np.asarray(cu_seqlens), T)
    key = (T, tuple(segments))
    if key not in _PROGRAM_CACHE:
        _PROGRAM_CACHE[key] = _build_program(T, segments)
    nc = _PROGRAM_CACHE[key]

    bf = ml_dtypes.bfloat16
    qh = np.ascontiguousarray(
        np.asarray(q).reshape(T, NKV * G, D).transpose(1, 2, 0)
    ).astype(bf)  # [16, 64, T]
    kh = np.ascontiguousarray(k_eff.reshape(T, NKV, D).transpose(1, 2, 0)).astype(bf)
    vh = v_eff.reshape(T, NKV, D).astype(bf)  # [T, 8, 64]

    in_maps = []
    for h in range(NKV):
        qk = np.empty((128, 2 * T), dtype=bf)
        qk[:, 0:T] = qh[2 * h : 2 * h + 2].reshape(128, T)
        qk[0:64, T : 2 * T] = kh[h]
        qk[64:128, T : 2 * T] = kh[h]
        in_maps.append({"qk": qk, "v": np.ascontiguousarray(vh[:, h, :])})

    res = run_bass_kernel_spmd(nc, in_maps, core_ids=list(range(8)), trace=TRACE)
    LAST_RESULT = res

    out = np.empty((T, NKV * G * D), dtype=np.float32)
    ov = out.reshape(T, NKV, G * D)
    for h in range(NKV):
        ov[:, h, :] = res.results[h]["o"]
    return out


# revision 3
# speedup vs baseline: 1.0100x; 1.0100x over previous
"""Paged-KV varlen causal GQA attention for Trainium2, sharded over 8 NeuronCores.

Problem (hardcoded from spec): T=4096 tokens, 16 q heads / 8 kv heads, head_dim=64,
scale=0.125. k/v are scattered into paged caches via slot_mapping, read back, and
causal varlen attention (segments from cu_seqlens) is computed.

Sharding: tensor-parallel over kv heads -- core h gets kv head h and its 2 GQA
query heads. slot_mapping / cu_seqlens handled on host (index math only).

Device kernel (per core), per (segment, q-head), score layout sT[keys, queries]:
  scores are packed tightly into wide PSUM tiles (up to 1536 f32 cols = 3 banks)
  so the ScalarE exp -- the serial bottleneck at 1 col/cycle -- runs in as few,
  as wide instructions as possible.
    sT[k, q] = kT.T @ qT                (PE, per 128-key tile, causal spans only)
    sT[diag] += Mtri.T @ I              (PE; adds -240 above the diagonal so the
                                         exp kills masked entries -- no VectorE
                                         mask multiply needed)
    w = exp(0.125 * sT)                 (ScalarE, PSUM->SBUF bf16, one exp/tile)
    o_acc[q, 0:65] += w.T @ [v | 1]     (PE: w slices are the stationary weights,
                                         65-wide moving v => 65 cycles per
                                         (key-tile, 128-query block); col 64 is
                                         the softmax denominator)
    o[q, :] = o_acc[q, 0:64] * rcp(den) (VectorE reciprocal + broadcast mul)
"""

import os
from math import ceil

import numpy as np
import ml_dtypes

import concourse.bass as bass
import concourse.mybir as mybir
import concourse.tile as tile
from concourse import bacc
from concourse.bass_utils import run_bass_kernel_spmd
from concourse.masks import make_identity

NKV = 8
G = 2
D = 64
SCALE = 0.125
NEG = -240.0  # NEG*SCALE = -30; exp(-30) ~ 1e-13

TRACE = bool(int(os.environ.get("KERNEL_TRACE", "0")))
LAST_RESULT = None

_PROGRAM_CACHE = {}


def _pack_tiles(L, widths):
    """Pack the causal spans of all 128-key tiles of one (segment, head) into
    score tiles of the given widths. Returns [(width, [(kt, qlo, qhi, off)])].
    All quantities are multiples of 128."""
    total = sum(L - 128 * kt for kt in range(L // 128))
    assert sum(widths) == total, (sum(widths), total)
    tiles = []
    cur = []
    used = 0
    wi = 0
    for kt in range(L // 128):
        pos = 128 * kt
        while pos < L:
            take = min(L - pos, widths[wi] - used)
            cur.append((kt, pos, pos + take, used))
            used += take
            pos += take
            if used == widths[wi]:
                tiles.append((widths[wi], cur))
                cur = []
                used = 0
                wi += 1
    assert not cur
    return tiles


def _build_program(T, segments):
    f32 = mybir.dt.float32
    bf16 = mybir.dt.bfloat16
    Exp = mybir.ActivationFunctionType.Exp

    nc = bacc.Bacc(
        "TRN2",
        target_bir_lowering=False,
        debug=False,
        enable_asserts=False,
        num_devices=8,
    )
    # qk_d[:, 0:T] = qT (two heads stacked on partitions); qk_d[:, T:2T] = kT
    # duplicated on both partition halves -- one DMA covers a head's q AND k
    qk_d = nc.dram_tensor("qk", [128, 2 * T], bf16, kind="ExternalInput").ap()
    v_d = nc.dram_tensor("v", [T, D], bf16, kind="ExternalInput").ap()
    o_d = nc.dram_tensor("o", [T, 2 * D], f32, kind="ExternalOutput").ap()

    NKT = T // 128  # key tiles over all segments

    with tile.TileContext(nc) as tc:
        import contextlib

        ctx = contextlib.ExitStack()
        const = ctx.enter_context(tc.tile_pool(name="const", bufs=1))
        qkpool = ctx.enter_context(tc.tile_pool(name="qk", bufs=1))
        vpool = ctx.enter_context(tc.tile_pool(name="vt", bufs=1))
        spool = ctx.enter_context(tc.tile_pool(name="se", bufs=7))
        opool = ctx.enter_context(tc.tile_pool(name="ob", bufs=2))
        ps_sc = ctx.enter_context(tc.tile_pool(name="ps_sc", bufs=2, space="PSUM"))
        ps_o = ctx.enter_context(tc.tile_pool(name="ps_o", bufs=1, space="PSUM"))

        # warmup activation: forces the Exp table load at t~0, overlapping the
        # input DMAs instead of delaying the first real exp
        wsrc = const.tile([128, 8], bf16)
        nc.gpsimd.memset(wsrc, 0.0)
        nc.scalar.activation(wsrc, wsrc, Exp, scale=SCALE)

        # Mtri[r, c] = NEG if c > r else 0; accumulated into the diagonal
        # 128-block of each score tile via I128 it adds NEG above the diagonal
        Mtri = const.tile([128, 128], bf16)
        nc.gpsimd.memset(Mtri, 0.0)
        nc.gpsimd.affine_select(
            out=Mtri,
            in_=Mtri,
            compare_op=mybir.AluOpType.is_gt,
            fill=NEG,
            base=1,
            pattern=[[-1, 128]],
            channel_multiplier=1,
        )
        I128 = const.tile([128, 128], bf16)
        make_identity(nc, I128)

        # qkt[:, 0:T] = qT; qkt[:, T:2T] = kT duplicated on both halves so
        # head h's QK contracts on PE rows [64h, 64h+64) with matching
        # lhsT/rhs base partitions
        qkt = qkpool.tile([128, 2 * T], bf16)
        qT = qkt[:, 0:T]
        kT = qkt[:, T : 2 * T]
        vst = vpool.tile([128, NKT, D + 1], bf16)

        def dma_qk(rows, c0, c1, eng=None):
            # one DMA for q cols [c0,c1) AND k cols [c0,c1) of a head half
            # (regular 3-level AP: two chunks at stride T)
            r0, r1 = rows
            (eng or nc.sync).dma_start(
                qkt[r0:r1].rearrange("p (c n) -> p c n", c=2)[:, :, c0:c1],
                qk_d[r0:r1].rearrange("p (c n) -> p c n", c=2)[:, :, c0:c1],
            )

        # --- input DMAs, all upfront (dep tracking is byte-range based) ---
        for si, (s0, s1) in enumerate(segments):
            if si == 0 and s1 - s0 >= 1024:
                # ladder: tiny k then q chunks so the first QK unblocks ~2.9us
                # in instead of waiting for full-segment transfers
                # laddered q+k chunks: the first exp needs only chunk 1
                dma_qk((0, 64), s0, s0 + 256)
                dma_qk((0, 64), s0 + 256, s0 + 768)
                dma_qk((0, 64), s0 + 768, s1)
                g0 = s0 // 128
                ng = (s1 - s0) // 128
                # v and head 1 ride the Pool SWDGE queue: its generation runs
                # on the idle Pool engine, landing them ~2us earlier than
                # queueing behind head 0's transfers on SP
                nc.gpsimd.dma_start(
                    vst[:, g0 : g0 + ng, 0:D],
                    v_d[s0:s1, :].rearrange("(n p) d -> p n d", p=128),
                )
                dma_qk((64, 128), s0, s1, eng=nc.gpsimd)
            else:
                g0 = s0 // 128
                ng = (s1 - s0) // 128
                dma_qk((0, 64), s0, s1)
                # non-critical later transfers ride the idle Pool SWDGE queue
                # (input DMAs have no waits, so its sequencer holds are short)
                dma_qk((64, 128), s0, s1, eng=nc.gpsimd)
                nc.gpsimd.dma_start(
                    vst[:, g0 : g0 + ng, 0:D],
                    v_d[s0:s1, :].rearrange("(n p) d -> p n d", p=128),
                )
            nc.gpsimd.memset(vst[:, g0 : g0 + ng, D : D + 1], 1.0)

        # --- work list: one entry per score tile ---
        work = []
        for si, (s0, s1) in enumerate(segments):
            L = s1 - s0
            assert L % 128 == 0
            total = sum(L - 128 * kt for kt in range(L // 128))
            for h in range(G):
                first_it = si == 0 and h == 0
                last_it = si == len(segments) - 1 and h == G - 1
                if first_it and L == 1024:
                    # ramp: tile 0 (all of kt0) gets four 256-wide sub-exps --
                    # the first one only needs the first small q/k DMA, so the
                    # ScalarE stream starts ~1us earlier; sub-exps share one
                    # PSUM buffer (disjoint column ranges, no rotation WAR)
                    widths = [1024, 640, 1408, 1536]
                    assert sum(widths) == total
                elif last_it and L == 1024:
                    # ramp down: kt6/kt7 diagonals land in narrow final tiles
                    # so the closing exp->PV->normalize->DMA chain is short
                    widths = [1536, 1536, 1024, 384, 128]
                else:
                    widths = []
                rem = total - sum(widths)
                while rem > 0:
                    w = min(1536, rem)
                    widths.append(w)
                    rem -= w
                work.append((si, s0, L, h, _pack_tiles(L, widths)))

        state = {}  # (si, h) -> dict(oacc=, rcp=)
        osb_by_seg = {}

        def emit_qk(si, s0, h, ti, width, pieces, splits=None):
            sc = ps_sc.tile([128, 1536], f32, tag="sc", name=f"sc_{si}_{h}_{ti}")
            se = spool.tile([128, 1536], bf16, tag="se", name=f"se_{si}_{h}_{ti}")
            sa = 0
            for sw in splits or [width]:
                sb = sa + sw
                for kt, qlo, qhi, off in pieces:
                    # clip the piece to this sub-exp's column range [sa, sb)
                    a = max(off, sa)
                    b = min(off + (qhi - qlo), sb)
                    if a >= b:
                        continue
                    klo = 128 * kt
                    diag = qlo == klo and a == off
                    c = a
                    while c < b:
                        # chunks split at PSUM bank boundaries AND at the end
                        # of the diagonal block, so each accumulation group's
                        # region is closed exactly by its last matmul
                        e = min(b, (c // 512 + 1) * 512)
                        is_diag = diag and c == a
                        if is_diag:
                            e = min(e, off + 128)
                        qa = s0 + qlo + (c - off)
                        nc.tensor.matmul(
                            sc[:, c:e],
                            kT[64 * h : 64 * h + 64, s0 + klo : s0 + klo + 128],
                            qT[64 * h : 64 * h + 64, qa : qa + (e - c)],
                            start=True,
                            stop=not is_diag,
                        )
                        if is_diag:
                            nc.tensor.matmul(
                                sc[:, off : off + 128],
                                Mtri[:, 0:128],
                                I128[:, 0:128],
                                start=False,
                                stop=True,
                            )
                        c = e
                nc.scalar.activation(se[:, sa:sb], sc[:, sa:sb], Exp, scale=SCALE)
                sa = sb
            return se

        def fin(si, s0, L, h, q0, nq, loc=None):
            """Normalize query blocks [q0, q0+nq) and, on the last head, DMA
            those 128*nq output rows (both heads' columns) to HBM. `loc`
            overrides the (accumulator tile, block) the data lives in."""
            st = state[(si, h)]
            oacc = st["oacc"]
            if st["rcp"] is None:
                st["rcp"] = opool.tile([128, 8], f32, tag="rcp", name=f"rcp_{si}_{h}")
            rcp = st["rcp"]
            if si not in osb_by_seg:
                osb_by_seg[si] = opool.tile(
                    [128, 8, 2 * D], f32, tag="osb", name=f"osb_{si}"
                )
            osb = osb_by_seg[si]
            tidx, blk = loc if loc is not None else (q0 // 4, q0 % 4)
            ot = oacc[tidx]
            lo = 128 * blk
            nc.vector.reciprocal(
                rcp[:, q0 : q0 + nq],
                ot[:, lo + D : lo + D + 128 * (nq - 1) + 1 : 128],
            )
            ov = ot[:, lo : lo + 128 * nq].rearrange("p (c d) -> p c d", d=128)[
                :, :, 0:D
            ]
            rv = rcp[:, q0 : q0 + nq].rearrange("p (c d) -> p c d", d=1)
            rv2, ov2 = bass.broadcast_tensor_aps(rv, ov)
            nc.vector.tensor_mul(osb[:, q0 : q0 + nq, D * h : D * h + D], ov2, rv2)
            if h == G - 1:
                nc.sync.dma_start(
                    o_d[s0 + 128 * q0 : s0 + 128 * (q0 + nq), :].rearrange(
                        "(c p) d -> p c d", p=128
                    ),
                    osb[:, q0 : q0 + nq, :],
                )

        def emit_pv_block(it_key, qt):
            """All PV matmuls for query block qt of iteration it_key, in kt
            order -- each PSUM bank sees strictly sequential accumulation
            groups (one open group per bank, a hardware/CoreSim constraint)."""
            si, s0, L, h, pieces_of, se_of = it_key
            key = (si, h)
            if key not in state:
                state[key] = {
                    "oacc": [
                        ps_o.tile(
                            [128, 512], f32, tag=f"oacc{j}", name=f"oacc_{si}_{h}_{j}"
                        )
                        for j in range((L // 128 + 3) // 4)
                    ],
                    "rcp": None,
                }
            oacc = state[key]["oacc"]
            g0 = s0 // 128
            nq_total = L // 128
            last_it = si == len(segments) - 1 and h == G - 1
            if last_it and nq_total == 8 and qt == 7:
                # the final query block accumulates in the (long-idle) lo
                # tile: its normalize runs in parallel with fin(4..6) on the
                # hi tile, and the closing DMA carries only 128 rows
                tidx, blk = 0, 0
            else:
                tidx, blk = qt // 4, qt % 4
            out = oacc[tidx][:, 128 * blk : 128 * blk + D + 1]
            for kt in range(qt + 1):
                gti, qlo, off = pieces_of[(kt, qt)]
                rel = off + (128 * qt - qlo)
                nc.tensor.matmul(
                    out,
                    se_of[gti][:, rel : rel + 128],
                    vst[:, g0 + kt, :],
                    start=(kt == 0),
                    stop=(kt == qt),
                )
            if last_it and nq_total == 8:
                if qt == 3:
                    fin(si, s0, L, h, 0, 4)
                elif qt == 6:
                    fin(si, s0, L, h, 4, 3)
                elif qt == 7:
                    fin(si, s0, L, h, 7, 1, loc=(0, 0))
            elif qt % 4 == 3 or qt == nq_total - 1:
                q0 = (qt // 4) * 4
                fin(si, s0, L, h, q0, min(4, nq_total - q0))

        # Emission: QK+exp stream per score tile; each query block's PV burst
        # is emitted LAG tiles after the tile holding its diagonal, so by the
        # time PV instructions reach the PE sequencer their exp has long
        # completed -- they never camp in the 4-deep PE wait queue blocking
        # the next QK's dispatch, and the ScalarE exp stream stays gapless.
        LAG = 3
        pending = []  # (global_tile_idx_of_diag, it_key, qt)
        gidx = 0
        for si, s0, L, h, tiles in work:
            pieces_of = {}
            diag_tile = {}
            for ti, (width, pieces) in enumerate(tiles):
                for kt, qlo, qhi, off in pieces:
                    for qt in range(qlo // 128, qhi // 128):
                        pieces_of[(kt, qt)] = (gidx + ti, qlo, off)
                    if qlo == 128 * kt:
                        diag_tile[kt] = gidx + ti
            se_of = {}
            it_key = (si, s0, L, h, pieces_of, se_of)
            for ti, (width, pieces) in enumerate(tiles):
                splits = (
                    [256, 256, 256, 256]
                    if (si == 0 and h == 0 and ti == 0 and width == 1024)
                    else None
                )
                se_of[gidx + ti] = emit_qk(si, s0, h, ti, width, pieces, splits)
                for qt in sorted(diag_tile):
                    if diag_tile[qt] == gidx + ti:
                        pending.append((gidx + ti, it_key, qt))
                while pending and pending[0][0] <= gidx + ti - LAG:
                    _, ik, qt = pending.pop(0)
                    emit_pv_block(ik, qt)
            gidx += len(tiles)
        for _, ik, qt in pending:
            emit_pv_block(ik, qt)

        ctx.close()

    nc.compile()
    return nc


def _segments_from_cu(cu_seqlens, T):
    edges = sorted(set([0, T] + [int(c) for c in cu_seqlens if 0 < int(c) < T]))
    return [(edges[i], edges[i + 1]) for i in range(len(edges) - 1)]


def kernel(q, k, v, k_cache, v_cache, slot_mapping, cu_seqlens):
    global LAST_RESULT
    T = q.shape[0]
    nslots = k_cache.shape[0]

    # Emulate scatter-then-gather through the paged cache: for duplicate slots
    # the last writer wins, so token i reads back k[lastw[slot[i]]].
    slot = np.asarray(slot_mapping, dtype=np.int64)
    lastw = np.zeros(nslots, dtype=np.int64)
    lastw[slot] = np.arange(T)
    lw = lastw[slot]
    k_eff = np.asarray(k)[lw]
    v_eff = np.asarray(v)[lw]

    segments = _segments_from_cu(np.asarray(cu_seqlens), T)
    key = (T, tuple(segments))
    if key not in _PROGRAM_CACHE:
        _PROGRAM_CACHE[key] = _build_program(T, segments)
    nc = _PROGRAM_CACHE[key]

    bf = ml_dtypes.bfloat16
    qh = np.ascontiguousarray(
        np.asarray(q).reshape(T, NKV * G, D).transpose(1, 2, 0)
    ).astype(bf)  # [16, 64, T]
    kh = np.ascontiguousarray(k_eff.reshape(T, NKV, D).transpose(1, 2, 0)).astype(bf)
    vh = v_eff.reshape(T, NKV, D).astype(bf)  # [T, 8, 64]

    in_maps = []
    for h in range(NKV):
        qk = np.empty((128, 2 * T), dtype=bf)
        qk[:, 0:T] = qh[2 * h : 2 * h + 2].reshape(128, T)
        qk[0:64, T : 2 * T] = kh[h]
        qk[64:128, T : 2 * T] = kh[h]
        in_maps.append({"qk": qk, "v": np.ascontiguousarray(vh[:, h, :])})

    res = run_bass_kernel_spmd(nc, in_maps, core_ids=list(range(8)), trace=TRACE)
    LAST_RESULT = res

    out = np.empty((T, NKV * G * D), dtype=np.float32)
    ov = out.reshape(T, NKV, G * D)
    for h in range(NKV):
        ov[:, h, :] = res.results[h]["o"]
    return out


# revision 4
# speedup vs baseline: 1.0244x; 1.0143x over previous
"""Paged-KV varlen causal GQA attention for Trainium2, sharded over 8 NeuronCores.

Problem (hardcoded from spec): T=4096 tokens, 16 q heads / 8 kv heads, head_dim=64,
scale=0.125. k/v are scattered into paged caches via slot_mapping, read back, and
causal varlen attention (segments from cu_seqlens) is computed.

Sharding: tensor-parallel over kv heads -- core h gets kv head h and its 2 GQA
query heads. slot_mapping / cu_seqlens handled on host (index math only).

Device kernel (per core), per (segment, q-head), score layout sT[keys, queries]:
  scores are packed tightly into wide PSUM tiles (up to 1536 f32 cols = 3 banks)
  so the ScalarE exp -- the serial bottleneck at 1 col/cycle -- runs in as few,
  as wide instructions as possible.
    sT[k, q] = kT.T @ qT                (PE, per 128-key tile, causal spans only)
    sT[diag] += Mtri.T @ I              (PE; adds -240 above the diagonal so the
                                         exp kills masked entries -- no VectorE
                                         mask multiply needed)
    w = exp(0.125 * sT)                 (ScalarE, PSUM->SBUF bf16, one exp/tile)
    o_acc[q, 0:65] += w.T @ [v | 1]     (PE: w slices are the stationary weights,
                                         65-wide moving v => 65 cycles per
                                         (key-tile, 128-query block); col 64 is
                                         the softmax denominator)
    o[q, :] = o_acc[q, 0:64] * rcp(den) (VectorE reciprocal + broadcast mul)
"""

import os
from math import ceil

import numpy as np
import ml_dtypes

import concourse.bass as bass
import concourse.mybir as mybir
import concourse.tile as tile
from concourse import bacc
from concourse.bass_utils import run_bass_kernel_spmd
from concourse.masks import make_identity

NKV = 8
G = 2
D = 64
SCALE = 0.125
NEG = -240.0  # NEG*SCALE = -30; exp(-30) ~ 1e-13

TRACE = bool(int(os.environ.get("KERNEL_TRACE", "0")))
LAST_RESULT = None

_PROGRAM_CACHE = {}


def _pack_tiles(L, widths, pair_diags=False):
    """Pack the causal spans of all 128-key tiles of one (segment, head) into
    score tiles of the given widths. Returns [(width, [piece])] where piece is
    (kt, qlo, qhi, off) or, with pair_diags, ("pair", kta, ktb, off): the
    first 64 diagonal columns of kta on partitions 0:64 and of ktb on 64:128
    share one 64-column range -- reclaiming sub-diagonal exp columns."""
    nkt = L // 128
    if pair_diags:
        assert nkt % 2 == 0
        total = 64 * (nkt // 2) + sum(L - 128 * kt - 64 for kt in range(nkt))
        stream = [("pair", 2 * j, 2 * j + 1) for j in range(nkt // 2)]
        stream += [(kt, 128 * kt + 64, L) for kt in range(nkt)]
    else:
        total = sum(L - 128 * kt for kt in range(nkt))
        stream = [(kt, 128 * kt, L) for kt in range(nkt)]
    assert sum(widths) == total, (sum(widths), total)
    tiles = []
    cur = []
    used = 0
    wi = 0

    def flush_if_full():
        nonlocal cur, used, wi
        if used == widths[wi]:
            tiles.append((widths[wi], cur))
            cur = []
            used = 0
            wi += 1

    for entry in stream:
        if entry[0] == "pair":
            assert used + 64 <= widths[wi]
            cur.append(("pair", entry[1], entry[2], used))
            used += 64
            flush_if_full()
        else:
            kt, pos, end = entry
            while pos < end:
                take = min(end - pos, widths[wi] - used)
                cur.append((kt, pos, pos + take, used))
                used += take
                pos += take
                flush_if_full()
    assert not cur, cur
    return tiles


def _build_program(T, segments):
    f32 = mybir.dt.float32
    bf16 = mybir.dt.bfloat16
    Exp = mybir.ActivationFunctionType.Exp

    nc = bacc.Bacc(
        "TRN2",
        target_bir_lowering=False,
        debug=False,
        enable_asserts=False,
        num_devices=8,
    )
    # qk_d[:, 0:T] = qT (two heads stacked on partitions); qk_d[:, T:2T] = kT
    # duplicated on both partition halves -- one DMA covers a head's q AND k
    qk_d = nc.dram_tensor("qk", [128, 2 * T], bf16, kind="ExternalInput").ap()
    v_d = nc.dram_tensor("v", [T, D], bf16, kind="ExternalInput").ap()
    o_d = nc.dram_tensor("o", [T, 2 * D], f32, kind="ExternalOutput").ap()

    NKT = T // 128  # key tiles over all segments

    with tile.TileContext(nc) as tc:
        import contextlib

        ctx = contextlib.ExitStack()
        const = ctx.enter_context(tc.tile_pool(name="const", bufs=1))
        qkpool = ctx.enter_context(tc.tile_pool(name="qk", bufs=1))
        vpool = ctx.enter_context(tc.tile_pool(name="vt", bufs=1))
        spool = ctx.enter_context(tc.tile_pool(name="se", bufs=7))
        opool = ctx.enter_context(tc.tile_pool(name="ob", bufs=2))
        ps_sc = ctx.enter_context(tc.tile_pool(name="ps_sc", bufs=2, space="PSUM"))
        ps_o = ctx.enter_context(tc.tile_pool(name="ps_o", bufs=1, space="PSUM"))

        # warmup activation: forces the Exp table load at t~0, overlapping the
        # input DMAs instead of delaying the first real exp
        wsrc = const.tile([128, 8], bf16)
        nc.gpsimd.memset(wsrc, 0.0)
        nc.scalar.activation(wsrc, wsrc, Exp, scale=SCALE)

        # Mtri[r, c] = NEG if c > r else 0; accumulated into the diagonal
        # 128-block of each score tile via I128 it adds NEG above the diagonal
        Mtri = const.tile([128, 128], bf16)
        nc.gpsimd.memset(Mtri, 0.0)
        nc.gpsimd.affine_select(
            out=Mtri,
            in_=Mtri,
            compare_op=mybir.AluOpType.is_gt,
            fill=NEG,
            base=1,
            pattern=[[-1, 128]],
            channel_multiplier=1,
        )
        I128 = const.tile([128, 128], bf16)
        make_identity(nc, I128)

        # qkt[:, 0:T] = qT; qkt[:, T:2T] = kT duplicated on both halves so
        # head h's QK contracts on PE rows [64h, 64h+64) with matching
        # lhsT/rhs base partitions
        qkt = qkpool.tile([128, 2 * T], bf16)
        qT = qkt[:, 0:T]
        kT = qkt[:, T : 2 * T]
        vst = vpool.tile([128, NKT, D + 1], bf16)
        # v rows replicated on partitions 64:128 for quadrant-packed PV
        vst2 = vpool.tile([128, NKT, D + 1], bf16)

        def dma_qk(rows, c0, c1, eng=None):
            # one DMA for q cols [c0,c1) AND k cols [c0,c1) of a head half
            # (regular 3-level AP: two chunks at stride T)
            r0, r1 = rows
            (eng or nc.sync).dma_start(
                qkt[r0:r1].rearrange("p (c n) -> p c n", c=2)[:, :, c0:c1],
                qk_d[r0:r1].rearrange("p (c n) -> p c n", c=2)[:, :, c0:c1],
            )

        # --- input DMAs, all upfront (dep tracking is byte-range based) ---
        for si, (s0, s1) in enumerate(segments):
            if si == 0 and s1 - s0 >= 1024:
                # ladder: tiny k then q chunks so the first QK unblocks ~2.9us
                # in instead of waiting for full-segment transfers
                # laddered q+k chunks: the first exp needs only chunk 1
                dma_qk((0, 64), s0, s0 + 256)
                dma_qk((0, 64), s0 + 256, s0 + 768)
                dma_qk((0, 64), s0 + 768, s1)
                g0 = s0 // 128
                ng = (s1 - s0) // 128
                # v and head 1 ride the Pool SWDGE queue: its generation runs
                # on the idle Pool engine, landing them ~2us earlier than
                # queueing behind head 0's transfers on SP
                nc.gpsimd.dma_start(
                    vst[:, g0 : g0 + ng, 0:D],
                    v_d[s0:s1, :].rearrange("(n p) d -> p n d", p=128),
                )
                dma_qk((64, 128), s0, s1, eng=nc.gpsimd)
            else:
                g0 = s0 // 128
                ng = (s1 - s0) // 128
                dma_qk((0, 64), s0, s1)
                # non-critical later transfers ride the idle Pool SWDGE queue
                # (input DMAs have no waits, so its sequencer holds are short)
                dma_qk((64, 128), s0, s1, eng=nc.gpsimd)
                nc.gpsimd.dma_start(
                    vst[:, g0 : g0 + ng, 0:D],
                    v_d[s0:s1, :].rearrange("(n p) d -> p n d", p=128),
                )
            nc.gpsimd.memset(vst[:, g0 : g0 + ng, D : D + 1], 1.0)
            nc.gpsimd.dma_start(
                vst2[64:128, g0 : g0 + ng, 0:D],
                v_d[s0:s1, :].rearrange("(n p) d -> p n d", p=128)[0:64],
            )
            nc.gpsimd.memset(vst2[64:128, g0 : g0 + ng, D : D + 1], 1.0)

        # --- work list: one entry per score tile ---
        work = []
        for si, (s0, s1) in enumerate(segments):
            L = s1 - s0
            assert L % 128 == 0
            total = sum(L - 128 * kt for kt in range(L // 128))
            for h in range(G):
                first_it = si == 0 and h == 0
                last_it = si == len(segments) - 1 and h == G - 1
                if first_it and L == 1024:
                    # ramp: tile 0 (all of kt0) gets four 256-wide sub-exps --
                    # the first one only needs the first small q/k DMA, so the
                    # ScalarE stream starts ~1us earlier; sub-exps share one
                    # PSUM buffer (disjoint column ranges, no rotation WAR)
                    widths = [1024, 640, 1408, 1536]
                    assert sum(widths) == total
                elif last_it and L == 1024:
                    # ramp down: kt6/kt7 diagonals land in narrow final tiles
                    # so the closing exp->PV->normalize->DMA chain is short
                    widths = [1536, 1536, 1024, 384, 128]
                else:
                    widths = []
                # quadrant (64,64) = PE quadrant 3 has a known HW bug, so
                # only head-0 iterations (whose QK-B lands in (64h=0, 64))
                # use diagonal pairing
                pair = not widths and L == 1024 and h == 0
                tot = total
                if pair:
                    tot = 64 * (L // 256) + sum(
                        L - 128 * kt - 64 for kt in range(L // 128)
                    )
                rem = tot - sum(widths)
                while rem > 0:
                    w = min(1536, rem)
                    widths.append(w)
                    rem -= w
                work.append((si, s0, L, h, _pack_tiles(L, widths, pair), pair))

        state = {}  # (si, h) -> dict(oacc=, rcp=)
        osb_by_seg = {}

        def emit_qk(si, s0, h, ti, width, pieces, splits=None):
            sc = ps_sc.tile([128, 1536], f32, tag="sc", name=f"sc_{si}_{h}_{ti}")
            se = spool.tile([128, 1536], bf16, tag="se", name=f"se_{si}_{h}_{ti}")
            sa = 0
            for sw in splits or [width]:
                sb = sa + sw
                for piece in pieces:
                    if piece[0] == "pair":
                        # two diag first-halves share one 64-col range:
                        # member a on partitions 0:64, member b on 64:128
                        _, kta, ktb, off = piece
                        if not (sa <= off < sb):
                            continue
                        for kt, p0 in ((kta, 0), (ktb, 64)):
                            ka = s0 + 128 * kt
                            nc.tensor.matmul(
                                sc[p0 : p0 + 64, off : off + 64],
                                kT[64 * h : 64 * h + 64, ka : ka + 64],
                                qT[64 * h : 64 * h + 64, ka : ka + 64],
                                start=True,
                                stop=False,
                            )
                            nc.tensor.matmul(
                                sc[p0 : p0 + 64, off : off + 64],
                                Mtri[0:64, 0:64],
                                I128[0:64, 0:64],
                                start=False,
                                stop=True,
                            )
                        continue
                    kt, qlo, qhi, off = piece
                    # clip the piece to this sub-exp's column range [sa, sb)
                    a = max(off, sa)
                    b = min(off + (qhi - qlo), sb)
                    if a >= b:
                        continue
                    klo = 128 * kt
                    shifted = qlo == klo + 64  # diag cols [klo+64, klo+128)
                    diag = (qlo == klo or shifted) and a == off
                    dw = 64 if shifted else 128
                    c = a
                    while c < b:
                        # chunks split at PSUM bank boundaries AND at the end
                        # of the diagonal block, so each accumulation group's
                        # region is closed exactly by its last matmul
                        e = min(b, (c // 512 + 1) * 512)
                        is_diag = diag and c == a
                        if is_diag:
                            e = min(e, off + dw)
                        qa = s0 + qlo + (c - off)
                        nc.tensor.matmul(
                            sc[:, c:e],
                            kT[64 * h : 64 * h + 64, s0 + klo : s0 + klo + 128],
                            qT[64 * h : 64 * h + 64, qa : qa + (e - c)],
                            start=True,
                            stop=not is_diag,
                        )
                        if is_diag:
                            nc.tensor.matmul(
                                sc[:, off : off + dw],
                                Mtri[:, 0:128],
                                I128[:, 128 - dw : 128],
                                start=False,
                                stop=True,
                            )
                        c = e
                nc.scalar.activation(se[:, sa:sb], sc[:, sa:sb], Exp, scale=SCALE)
                sa = sb
            return se

        def fin(si, s0, L, h, q0, nq, loc=None):
            """Normalize query blocks [q0, q0+nq) and, on the last head, DMA
            those 128*nq output rows (both heads' columns) to HBM. `loc`
            overrides the (accumulator tile, block) the data lives in."""
            st = state[(si, h)]
            oacc = st["oacc"]
            if st["rcp"] is None:
                st["rcp"] = opool.tile([128, 8], f32, tag="rcp", name=f"rcp_{si}_{h}")
            rcp = st["rcp"]
            if si not in osb_by_seg:
                osb_by_seg[si] = opool.tile(
                    [128, 8, 2 * D], f32, tag="osb", name=f"osb_{si}"
                )
            osb = osb_by_seg[si]
            tidx, blk = loc if loc is not None else (q0 // 4, q0 % 4)
            ot = oacc[tidx]
            lo = 128 * blk
            nc.vector.reciprocal(
                rcp[:, q0 : q0 + nq],
                ot[:, lo + D : lo + D + 128 * (nq - 1) + 1 : 128],
            )
            ov = ot[:, lo : lo + 128 * nq].rearrange("p (c d) -> p c d", d=128)[
                :, :, 0:D
            ]
            rv = rcp[:, q0 : q0 + nq].rearrange("p (c d) -> p c d", d=1)
            rv2, ov2 = bass.broadcast_tensor_aps(rv, ov)
            nc.vector.tensor_mul(osb[:, q0 : q0 + nq, D * h : D * h + D], ov2, rv2)
            if h == G - 1:
                nc.sync.dma_start(
                    o_d[s0 + 128 * q0 : s0 + 128 * (q0 + nq), :].rearrange(
                        "(c p) d -> p c d", p=128
                    ),
                    osb[:, q0 : q0 + nq, :],
                )

        def emit_pv_block(it_key, qt):
            """All PV matmuls for query block qt of iteration it_key, in kt
            order -- each PSUM bank sees strictly sequential accumulation
            groups (one open group per bank, a hardware/CoreSim constraint)."""
            si, s0, L, h, pieces_of, se_of = it_key[:6]
            key = (si, h)
            if key not in state:
                state[key] = {
                    "oacc": [
                        ps_o.tile(
                            [128, 512], f32, tag=f"oacc{j}", name=f"oacc_{si}_{h}_{j}"
                        )
                        for j in range((L // 128 + 3) // 4)
                    ],
                    "rcp": None,
                }
            oacc = state[key]["oacc"]
            g0 = s0 // 128
            nq_total = L // 128
            last_it = si == len(segments) - 1 and h == G - 1
            if last_it and nq_total == 8 and qt == 7:
                # the final query block accumulates in the (long-idle) lo
                # tile: its normalize runs in parallel with fin(4..6) on the
                # hi tile, and the closing DMA carries only 128 rows
                tidx, blk = 0, 0
            else:
                tidx, blk = qt // 4, qt % 4
            pairloc, halfdiag = it_key[6], it_key[7]
            out = oacc[tidx][:, 128 * blk : 128 * blk + D + 1]
            paired = qt in pairloc
            nfull = qt + 1 - (1 if paired else 0)
            for kt in range(nfull):
                gti, qlo, off = pieces_of[(kt, qt)]
                rel = off + (128 * qt - qlo)
                nc.tensor.matmul(
                    out,
                    se_of[gti][:, rel : rel + 128],
                    vst[:, g0 + kt, :],
                    start=(kt == 0),
                    # paired: the last full PV closes the bank's group; the
                    # row-split PVs below are exempt from group bookkeeping
                    stop=(kt == qt) or (paired and kt == nfull - 1),
                )
            if paired:
                # qt's own tile, rows 64:128 from the shifted diag piece
                hg, hoff = halfdiag[qt]
                nc.tensor.matmul(
                    oacc[tidx][64:128, 128 * blk : 128 * blk + D + 1],
                    se_of[hg][:, hoff : hoff + 64],
                    vst[:, g0 + qt, :],
                    start=(qt == 0),
                    stop=True,
                    skip_group_check=True,
                )
                # rows 0:64 from the packed pair quadrant (64-key contraction)
                pg, poff, side = pairloc[qt]
                vs = vst if side == 0 else vst2
                nc.tensor.matmul(
                    oacc[tidx][0:64, 128 * blk : 128 * blk + D + 1],
                    se_of[pg][64 * side : 64 * side + 64, poff : poff + 64],
                    vs[64 * side : 64 * side + 64, g0 + qt, :],
                    start=(qt == 0),
                    stop=True,
                    skip_group_check=True,
                )
            if last_it and nq_total == 8:
                if qt == 3:
                    fin(si, s0, L, h, 0, 4)
                elif qt == 6:
                    fin(si, s0, L, h, 4, 3)
                elif qt == 7:
                    fin(si, s0, L, h, 7, 1, loc=(0, 0))
            elif qt % 4 == 3 or qt == nq_total - 1:
                q0 = (qt // 4) * 4
                fin(si, s0, L, h, q0, min(4, nq_total - q0))

        # Emission: QK+exp stream per score tile; each query block's PV burst
        # is emitted LAG tiles after the tile holding its diagonal, so by the
        # time PV instructions reach the PE sequencer their exp has long
        # completed -- they never camp in the 4-deep PE wait queue blocking
        # the next QK's dispatch, and the ScalarE exp stream stays gapless.
        LAG = 3
        pending = []  # (global_tile_idx_of_diag, it_key, qt)
        gidx = 0
        for si, s0, L, h, tiles, pair in work:
            pieces_of = {}
            diag_tile = {}
            pairloc = {}
            halfdiag = {}
            for ti, (width, pieces) in enumerate(tiles):
                for piece in pieces:
                    if piece[0] == "pair":
                        _, kta, ktb, off = piece
                        pairloc[kta] = (gidx + ti, off, 0)
                        pairloc[ktb] = (gidx + ti, off, 1)
                        for kt in (kta, ktb):
                            diag_tile[kt] = max(diag_tile.get(kt, 0), gidx + ti)
                        continue
                    kt, qlo, qhi, off = piece
                    for qt in range((qlo + 127) // 128, qhi // 128):
                        pieces_of[(kt, qt)] = (gidx + ti, qlo, off)
                    if qlo == 128 * kt:
                        diag_tile[kt] = max(diag_tile.get(kt, 0), gidx + ti)
                    elif qlo == 128 * kt + 64:
                        halfdiag[kt] = (gidx + ti, off)
                        diag_tile[kt] = max(diag_tile.get(kt, 0), gidx + ti)
            se_of = {}
            it_key = (si, s0, L, h, pieces_of, se_of, pairloc, halfdiag)
            for ti, (width, pieces) in enumerate(tiles):
                splits = (
                    [256, 256, 256, 256]
                    if (si == 0 and h == 0 and ti == 0 and width == 1024)
                    else None
                )
                se_of[gidx + ti] = emit_qk(si, s0, h, ti, width, pieces, splits)
                for qt in sorted(diag_tile):
                    if diag_tile[qt] == gidx + ti:
                        pending.append((gidx + ti, it_key, qt))
                while pending and pending[0][0] <= gidx + ti - LAG:
                    _, ik, qt = pending.pop(0)
                    emit_pv_block(ik, qt)
            gidx += len(tiles)
        for _, ik, qt in pending:
            emit_pv_block(ik, qt)

        ctx.close()

    nc.compile()
    return nc


def _segments_from_cu(cu_seqlens, T):
    edges = sorted(set([0, T] + [int(c) for c in cu_seqlens if 0 < int(c) < T]))
    return [(edges[i], edges[i + 1]) for i in range(len(edges) - 1)]


def kernel(q, k, v, k_cache, v_cache, slot_mapping, cu_seqlens):
    global LAST_RESULT
    T = q.shape[0]
    nslots = k_cache.shape[0]

    # Emulate scatter-then-gather through the paged cache: for duplicate slots
    # the last writer wins, so token i reads back k[lastw[slot[i]]].
    slot = np.asarray(slot_mapping, dtype=np.int64)
    lastw = np.zeros(nslots, dtype=np.int64)
    lastw[slot] = np.arange(T)
    lw = lastw[slot]
    k_eff = np.asarray(k)[lw]
    v_eff = np.asarray(v)[lw]

    segments = _segments_from_cu(np.asarray(cu_seqlens), T)
    key = (T, tuple(segments))
    if key not in _PROGRAM_CACHE:
        _PROGRAM_CACHE[key] = _build_program(T, segments)
    nc = _PROGRAM_CACHE[key]

    bf = ml_dtypes.bfloat16
    qh = np.ascontiguousarray(
        np.asarray(q).reshape(T, NKV * G, D).transpose(1, 2, 0)
    ).astype(bf)  # [16, 64, T]
    kh = np.ascontiguousarray(k_eff.reshape(T, NKV, D).transpose(1, 2, 0)).astype(bf)
    vh = v_eff.reshape(T, NKV, D).astype(bf)  # [T, 8, 64]

    in_maps = []
    for h in range(NKV):
        qk = np.empty((128, 2 * T), dtype=bf)
        qk[:, 0:T] = qh[2 * h : 2 * h + 2].reshape(128, T)
        qk[0:64, T : 2 * T] = kh[h]
        qk[64:128, T : 2 * T] = kh[h]
        in_maps.append({"qk": qk, "v": np.ascontiguousarray(vh[:, h, :])})

    res = run_bass_kernel_spmd(nc, in_maps, core_ids=list(range(8)), trace=TRACE)
    LAST_RESULT = res

    out = np.empty((T, NKV * G * D), dtype=np.float32)
    ov = out.reshape(T, NKV, G * D)
    for h in range(NKV):
        ov[:, h, :] = res.results[h]["o"]
    return out
